# revision 1
# baseline (speedup 1.0000x reference)
"""Distance-kernel multi-head attention on 8 TRN2 NeuronCores (Bass/Tile).

Problem: nn_MultiHeadAttention_80272938762455.

Math (per batch b, head h, S=2048, d_k=64):
    q = queries @ Wq.T, k = keys @ Wk.T, v = values @ Wv.T   (split to heads)
    d2[s,t]   = |q_s - k_t|^2
    compat    = (1 + sqrt(d2)/64) ** -65
    N_C[t]    = sum_s compat[s,t]
    M[s,t]    = compat[s,t] * N_C[t]^-1/2       (the N_R^-1/2 row factor of the
                Sinkhorn step cancels exactly in the row L1-normalization)
    vw        = M / rowsum(M)
    out       = concat_h(vw @ v_h) @ Wo.T + bo

Sharding: core i handles batch b = i//2 and head-half hh = i%2 (8 heads, model
dims 512*hh..512*hh+512).  Each core returns a partial [S, 1024] output
projection; the host sums the two partials per batch and adds bo.

Device pipeline per core (all elementwise transcendentals on the ACT engine
with a single natural_log_exp table set -- sqrt is computed as exp(0.5*ln)
because the hardware sqrt table is low precision and lives in another set):
    phase 1: project Q^T,K^T (d-major layout), V (s-major, fp16), per-head
             q^2 rows / k^2 columns via ones-matmuls.
    phase 2 per head: d2^T[t,s] on the PE (k.q - q2/2 in PSUM; the Ln pass
             applies scale=-2 and the per-partition k2 bias), then the 4-pass
             ACT chain Ln -> Exp -> Ln -> Exp producing compat^T in fp16
             scaled by 2^14 (free-dim accum_out yields N_C for free).  The
             attention matmul runs transposed, attnT[j,s] += vp_tt^T @ c_tt,
             t-outer so the previous head's attention interleaves with the
             next head's elementwise chain; each accumulation group owns a
             full PSUM bank (same-bank concurrent groups mis-accumulate on
             this stack).  attnT rows are normalized by a broadcast
             reciprocal of the trailing w-row and written straight into the
             m-partitioned mergedT tiles -- no transpose needed anywhere.
    phase 3: out_part[s,:] = mergedT^T @ woT on the PE.
"""

import math

import numpy as np

import concourse.bass as bass
import concourse.mybir as mybir
import concourse.tile as tile
from concourse.bass import ts
from concourse.bass_utils import run_bass_kernel_spmd
from concourse.vector_clock import ScopedClock

F32 = mybir.dt.float32
F16 = mybir.dt.float16
AF = mybir.ActivationFunctionType

S = 2048          # sequence length
D = 1024          # model dim
P = 128           # partitions
NT = S // P       # 16 t/s tiles
DCORE = 512       # head dims handled per core (8 heads x 64)
HCORE = 8         # heads per core
DK = 64
N_CORES = 8
B = 4

LN64 = math.log(64.0)
CBIAS = 14.0 * math.log(2.0)   # compat stored as 2^14 * compat (fp16 range)
NEXP = -65.0                   # -(d_intrinsic + alpha)


def _patch_tail_drain():
    """walrus codegen only accepts one sync-wait command per instruction;
    Tile's kernel-tail drain carries one wait per live proc.  Split it into
    a chain of single-wait drains."""
    if getattr(tile.TileContext, "_ant_drain_patched", False):
        return

    def _drain_and_barrier(self, tick_clock, wait_clock):
        nc = self.nc
        drain_inst = nc.sync.drain()
        wait_clock.add_sem_waits(
            drain_inst.ins, ScopedClock({None: tick_clock.global_clock})
        )
        waits = list(drain_inst.ins.sync_info.on_wait)
        if len(waits) > 1:
            drain_inst.ins.sync_info = mybir.SyncInfo(
                on_wait=waits[:1], on_update=[]
            )
            for w in waits[1:]:
                d2 = nc.sync.drain()
                d2.ins.sync_info = mybir.SyncInfo(on_wait=[w], on_update=[])
        nc.all_engine_barrier()
        popped = nc._tile_sem_poison_stack.pop()
        assert popped is self._sem_poison
        nc.clear_and_free_semaphores(list(self.sems.allocated().values()))
        nc.all_engine_barrier()

    tile.TileContext._drain_and_barrier = _drain_and_barrier
    tile.TileContext._ant_drain_patched = True


def _split_waits(nc):
    """This walrus build accepts at most ONE embedded sync-wait command per
    instruction.  Tile's sem-assignment freely emits several.  Splice
    single-wait Drains immediately in front of any instruction carrying more
    than one wait -- a serial queue waiting twice is semantically identical
    to one instruction waiting on both."""
    wid = 0
    for f in nc.m.functions:
        for bb in f.blocks:
            il = bb.instructions
            if not any(i.sync_info is not None
                       and len(i.sync_info.on_wait or []) > 1 for i in il):
                continue
            out = []
            for inst in il:
                si = inst.sync_info
                waits = list(si.on_wait) if si is not None and si.on_wait else []
                if len(waits) > 1:
                    for w in waits[:-1]:
                        nop = mybir.InstDrain(name=f"WS-{wid}",
                                              engine=inst.engine)
                        wid += 1
                        nop.sync_info = mybir.SyncInfo(on_wait=[w],
                                                       on_update=[])
                        out.append(nop)
                    inst.sync_info = mybir.SyncInfo(
                        on_wait=[waits[-1]],
                        on_update=list(si.on_update or []))
                out.append(inst)
            bb.instructions = out


def build_nc(dbg=False):
    _patch_tail_drain()
    nc = bass.Bass("TRN2", target_bir_lowering=False, debug=False,
                   num_devices=N_CORES)

    qT = nc.dram_tensor("qT", [D, S], F32, kind="ExternalInput").ap()
    kT = nc.dram_tensor("kT", [D, S], F32, kind="ExternalInput").ap()
    vT = nc.dram_tensor("vT", [D, S], F32, kind="ExternalInput").ap()
    wqT = nc.dram_tensor("wqT", [D, DCORE], F32, kind="ExternalInput").ap()
    wkT = nc.dram_tensor("wkT", [D, DCORE], F32, kind="ExternalInput").ap()
    wvT = nc.dram_tensor("wvT", [D, DCORE], F32, kind="ExternalInput").ap()
    woT = nc.dram_tensor("woT", [DCORE, D], F16, kind="ExternalInput").ap()
    out_part = nc.dram_tensor("out_part", [S, D], F32, kind="ExternalOutput").ap()
    rr_dram = nc.dram_tensor("rr_dram", [1, S], F32).ap()
    if dbg:
        dbg_c = nc.dram_tensor("dbg_c", [S, S], F16, kind="ExternalOutput").ap()
        dbg_mt = nc.dram_tensor("dbg_mt", [DCORE, S], F16, kind="ExternalOutput").ap()

    from contextlib import ExitStack
    with tile.TileContext(nc) as tc, ExitStack() as stack:
        persist = stack.enter_context(tc.tile_pool(name="persist", bufs=1))
        QT = [persist.tile([P, S], F32, name=f"QTd{d}") for d in range(4)]
        KT = [persist.tile([P, S], F32, name=f"KTd{d}") for d in range(4)]
        V = [persist.tile([P, DCORE], F16, name=f"Vs{sb}") for sb in range(NT)]
        mergedT = [persist.tile([P, S], F16, name=f"mT{mt}") for mt in range(4)]
        q2row = persist.tile([HCORE, S], F32, name="q2row")
        k2col = persist.tile([P, P], F32, name="k2col")   # col h*16+tt
        consts = persist.tile([P, 136], F32, name="consts")
        mutA = persist.tile([P, 32], F32, name="mutA")    # 0:16 N_C, 16:32 w

        # consts: col0 = 1 on parts 0-63, col1 = 1 on parts 64-127 (per-head
        # q^2 ones-matmul); col2/col3 = 1 on parts 0-63 / 64-127 (k^2 column
        # matmuls); row0 cols 4:132 = 1 (rank-1 q2 broadcast matmul lhsT).
        nc.vector.memset(consts, 0.0)
        nc.vector.memset(consts[0:64, 0:1], 1.0)
        nc.vector.memset(consts[64:128, 1:2], 1.0)
        nc.vector.memset(consts[0:64, 2:3], 1.0)
        nc.vector.memset(consts[64:128, 3:4], 1.0)
        nc.vector.memset(consts[0:1, 4:132], 1.0)
        nc.vector.memset(consts[:, 132:133], -LN64)   # Exp bias: g = e^(z/2)/64
        nc.vector.memset(consts[:, 133:134], CBIAS)   # Exp bias: 2^14 scale

        # ---------------- phase 1: projections -----------------------------
        with tc.tile_pool(name="xin", bufs=1) as xin, \
             tc.tile_pool(name="win", bufs=1) as win, \
             tc.tile_pool(name="sqp", bufs=1) as sqp:

            def load_inputs(src_dram, w_dram):
                xs, ws = [], []
                for dm in range(8):
                    x_t = xin.tile([P, S], F32, name=f"x{dm}", tag=f"x{dm}")
                    nc.sync.dma_start(out=x_t, in_=src_dram[dm * P:(dm + 1) * P, :])
                    w_t = win.tile([P, DCORE], F32, name=f"w{dm}", tag=f"w{dm}")
                    nc.sync.dma_start(out=w_t, in_=w_dram[dm * P:(dm + 1) * P, :])
                    xs.append(x_t)
                    ws.append(w_t)
                return xs, ws

            # Q then K: output d-major tiles [128 d, 2048 s]
            for which, (src, wsrc, XT) in enumerate(
                    [(qT, wqT, QT), (kT, wkT, KT)]):
                xs, ws = load_inputs(src, wsrc)
                with tc.tile_pool(name=f"ps{which}", bufs=1, space="PSUM") as pp, \
                     tc.tile_pool(name=f"ps2{which}", bufs=1, space="PSUM") as pp2:
                    for d in range(4):
                        ps = pp.tile([P, S], F32, name=f"proj{which}_{d}",
                                     tag="proj")
                        for dm in range(8):
                            for n in range(4):
                                nc.tensor.matmul(
                                    ps[:, ts(n, 512)],
                                    ws[dm][:, ts(d, P)],
                                    xs[dm][:, ts(n, 512)],
                                    start=(dm == 0), stop=(dm == 7))
                        nc.vector.tensor_copy(XT[d], ps)
                        sq = sqp.tile([P, S], F32, name=f"sq{which}_{d}",
                                      tag="sq")
                        nc.vector.tensor_mul(sq, XT[d], XT[d])
                        if which == 0:
                            # q^2 rows: [2, S] per d-tile via block-ones lhsT
                            q2ps = pp2.tile([2, S], F32, name=f"q2p{d}",
                                            tag="q2p")
                            for n in range(4):
                                nc.tensor.matmul(
                                    q2ps[:, ts(n, 512)], consts[:, 0:2],
                                    sq[:, ts(n, 512)], start=True, stop=True)
                            # store -q2/2: the d2 PSUM accumulates k.q - q2/2
                            # and the Ln pass applies scale=-2 plus the k2
                            # bias.  (engines can't address odd partition
                            # bases, so the rows go via an SBUF<->SBUF DMA)
                            q2st = sqp.tile([2, S], F32, name=f"q2st{d}",
                                            tag="q2st")
                            nc.vector.tensor_scalar_mul(q2st, q2ps, -0.5)
                            nc.sync.dma_start(out=q2row[2 * d:2 * d + 2, :],
                                              in_=q2st)
                        else:
                            # k^2 columns: [128,1] per (head, t-tile)
                            for p_ in range(2):
                                h = 2 * d + p_
                                off = 64 * p_
                                k2ps = pp2.tile([P, NT], F32, name=f"k2p{h}",
                                                tag="q2p")
                                ones_col = (consts[0:64, 2:3] if off == 0
                                            else consts[64:128, 3:4])
                                for tt in range(NT):
                                    nc.tensor.matmul(
                                        k2ps[:, tt:tt + 1],
                                        sq[off:off + 64, ts(tt, P)],
                                        ones_col,
                                        start=True, stop=True)
                                nc.vector.tensor_copy(
                                    k2col[:, h * NT:(h + 1) * NT], k2ps)

            # V: output s-major fp16 tiles [128 s, 512 d]
            xs, ws = load_inputs(vT, wvT)
            with tc.tile_pool(name="psv", bufs=2, space="PSUM") as ppv:
                for sb in range(NT):
                    vps = ppv.tile([P, DCORE], F32, name=f"vps{sb}", tag="vps")
                    for dm in range(8):
                        nc.tensor.matmul(vps, xs[dm][:, ts(sb, P)], ws[dm],
                                         start=(dm == 0), stop=(dm == 7))
                    nc.vector.tensor_copy(V[sb], vps)

        # ---------------- phase 2: per-head kernel + attention --------------
        with tc.tile_pool(name="compat", bufs=1) as cpool, \
             tc.tile_pool(name="upool", bufs=1) as upool, \
             tc.tile_pool(name="qaugp", bufs=1) as qaugp, \
             tc.tile_pool(name="vpp", bufs=1) as vpp, \
             tc.tile_pool(name="rrp", bufs=1) as rrp, \
             tc.tile_pool(name="d2ps", bufs=1, space="PSUM") as d2ps, \
             tc.tile_pool(name="atps", bufs=1, space="PSUM") as atps:

            prev = None  # (compat tiles, vp tile, attnT psum, head index)
            for h in range(HCORE + 1):
                cur_tiles = None
                vp = None
                if h < HCORE:
                    d, off = h // 2, 64 * (h % 2)
                    qaug = qaugp.tile([1, S], F32, name=f"qaug{h}", tag="qa")
                    nc.sync.dma_start(out=qaug, in_=q2row[h:h + 1, :])
                    cur_tiles = []

                for tt in range(NT):
                    if h < HCORE:
                        ps2 = d2ps.tile([P, S], F32, name=f"d2_{h}_{tt}",
                                        tag="d2")
                        for n in range(4):
                            nc.tensor.matmul(
                                ps2[:, ts(n, 512)],
                                KT[d][off:off + 64, ts(tt, P)],
                                QT[d][off:off + 64, ts(n, 512)],
                                start=True, stop=False)
                        for n in range(4):
                            nc.tensor.matmul(
                                ps2[:, ts(n, 512)],
                                consts[0:1, 4:132],
                                qaug[0:1, ts(n, 512)],
                                start=False, stop=True)
                    if prev is not None:
                        pc, pvp, pat, _ph = prev
                        # attnT[j, s] += vp_tt^T @ c_tt; one accumulation
                        # group per PSUM bank (n-chunk)
                        for n in range(4):
                            nc.tensor.matmul(
                                pat[0:65, ts(n, 512)],
                                pvp[:, tt, 0:65],
                                pc[tt][:, ts(n, 512)],
                                start=(tt == 0), stop=(tt == NT - 1))
                    if h < HCORE:
                        # z = ln(d2): d2 = -2*(k.q - q2/2) + k2 via scale+bias
                        nc.scalar.activation(
                            out=ps2, in_=ps2, func=AF.Ln, scale=-2.0,
                            bias=k2col[:, h * NT + tt:h * NT + tt + 1])
                        # g = sqrt(d2)/64 = exp(0.5 z - ln 64)
                        nc.scalar.activation(out=ps2, in_=ps2, func=AF.Exp,
                                             scale=0.5,
                                             bias=consts[:, 132:133])
                        u = upool.tile([P, S], F32, name=f"u{h}_{tt}", tag="u")
                        nc.scalar.activation(out=u, in_=ps2, func=AF.Ln,
                                             bias=1.0)
                        ct = cpool.tile([P, S], F16, name=f"c{h}_{tt}",
                                        tag=f"c{tt}")
                        nc.scalar.activation(
                            out=ct, in_=u, func=AF.Exp, scale=NEXP,
                            bias=consts[:, 133:134],
                            accum_out=mutA[:, tt:tt + 1])
                        cur_tiles.append(ct)

                if dbg and h == 0:
                    for tt in range(NT):
                        nc.sync.dma_start(
                            out=dbg_c[tt * P:(tt + 1) * P, :],
                            in_=cur_tiles[tt])

                if h < HCORE:
                    # w = N_C^-1/2 (2^14 scale cancels in the normalization)
                    nc.scalar.activation(out=mutA[:, 16:32],
                                         in_=mutA[:, 0:16], func=AF.Ln)
                    nc.scalar.activation(out=mutA[:, 16:32],
                                         in_=mutA[:, 16:32], func=AF.Exp,
                                         scale=-0.5)
                    vp = vpp.tile([P, NT, 68], F16, name=f"vp{h}", tag="vp")
                    for tt in range(NT):
                        nc.vector.tensor_scalar_mul(
                            vp[:, tt, 0:DK],
                            V[tt][:, h * DK:(h + 1) * DK],
                            mutA[:, 16 + tt:17 + tt])
                        nc.vector.tensor_copy(vp[:, tt, DK:DK + 1],
                                              mutA[:, 16 + tt:17 + tt])

                if prev is not None:
                    _pc, _pvp, pat, ph = prev
                    # rrec = 1/R from the trailing w-row, broadcast over the
                    # 64 head-dim partitions, then one fused normalize-store
                    # into the m-partitioned mergedT tile.
                    rr = rrp.tile([1, S], F32, name=f"rr{ph}", tag="rr")
                    nc.scalar.activation(out=rr, in_=pat[64:65, :], func=AF.Ln)
                    nc.scalar.activation(out=rr, in_=rr, func=AF.Exp,
                                         scale=-1.0)
                    rrb = rrp.tile([64, S], F32, name=f"rrb{ph}", tag="rrb")
                    nc.sync.dma_start(out=rr_dram, in_=rr)
                    nc.sync.dma_start(out=rrb, in_=rr_dram.to_broadcast((64, S)))
                    mt, moff = ph // 2, 64 * (ph % 2)
                    nc.vector.tensor_mul(
                        mergedT[mt][moff:moff + 64, :], pat[0:64, :], rrb)

                if h < HCORE:
                    pat_new = atps.tile([P, S], F32, name=f"at{h}", tag="at")
                    prev = (cur_tiles, vp, pat_new, h)
                else:
                    prev = None

        if dbg:
            for mt in range(4):
                nc.sync.dma_start(out=dbg_mt[mt * P:(mt + 1) * P, :],
                                  in_=mergedT[mt])

        # ---------------- phase 3: output projection ------------------------
        with tc.tile_pool(name="wop", bufs=1) as wop, \
             tc.tile_pool(name="outs", bufs=3) as outs, \
             tc.tile_pool(name="ops", bufs=2, space="PSUM") as ops:
            wo = []
            for mt in range(4):
                w_t = wop.tile([P, D], F16, name=f"wo{mt}", tag=f"wo{mt}")
                nc.sync.dma_start(out=w_t, in_=woT[mt * P:(mt + 1) * P, :])
                wo.append(w_t)
            for sb in range(NT):
                po = ops.tile([P, D], F32, name=f"po{sb}", tag="po")
                for mt in range(4):
                    for n2 in range(2):
                        nc.tensor.matmul(po[:, ts(n2, 512)],
                                         mergedT[mt][:, ts(sb, P)],
                                         wo[mt][:, ts(n2, 512)],
                                         start=(mt == 0), stop=(mt == 3))
                ot = outs.tile([P, D], F32, name=f"ot{sb}", tag="ot")
                nc.vector.tensor_copy(ot, po)
                nc.sync.dma_start(out=out_part[sb * P:(sb + 1) * P, :], in_=ot)

    _split_waits(nc)
    return nc


_NC_CACHE = None


def _get_nc():
    global _NC_CACHE
    if _NC_CACHE is None:
        _NC_CACHE = build_nc()
    return _NC_CACHE


def kernel(queries, keys, values, Wq, Wk, Wv, Wo, bo, _trace=False):
    queries = np.ascontiguousarray(np.asarray(queries, dtype=np.float32))
    keys = np.ascontiguousarray(np.asarray(keys, dtype=np.float32))
    values = np.ascontiguousarray(np.asarray(values, dtype=np.float32))
    Wq = np.asarray(Wq, dtype=np.float32)
    Wk = np.asarray(Wk, dtype=np.float32)
    Wv = np.asarray(Wv, dtype=np.float32)
    Wo = np.asarray(Wo, dtype=np.float32)
    bo = np.asarray(bo, dtype=np.float32)

    in_maps = []
    for core in range(N_CORES):
        b, hh = core // 2, core % 2
        dims = slice(DCORE * hh, DCORE * hh + DCORE)
        in_maps.append({
            "qT": np.ascontiguousarray(queries[b].T),
            "kT": np.ascontiguousarray(keys[b].T),
            "vT": np.ascontiguousarray(values[b].T),
            "wqT": np.ascontiguousarray(Wq[dims, :].T),
            "wkT": np.ascontiguousarray(Wk[dims, :].T),
            "wvT": np.ascontiguousarray(Wv[dims, :].T),
            "woT": np.ascontiguousarray(Wo[:, dims].T.astype(np.float16)),
        })

    res = run_bass_kernel_spmd(_get_nc(), in_maps, list(range(N_CORES)),
                               trace=_trace)

    out = np.empty((B, S, D), dtype=np.float32)
    for b in range(B):
        out[b] = (res.results[2 * b]["out_part"]
                  + res.results[2 * b + 1]["out_part"] + bo)
    if _trace:
        kernel._last_results = res
    return out

